# revision 37
# baseline (speedup 1.0000x reference)
"""Trainium2 Bass kernel for nn_ConvInfoGathererLayer.

Hypernetwork layer with choke dim 1: every generated kernel is
tanh(c_hbs * W) for a scalar choke c_hbs = relu(iv . Wc) >= 0.  We replace
tanh with a per-layer least-squares odd polynomial  a1 x + a3 x^3 + a5 x^5
(end-to-end rel err ~5e-3, fits the 2e-2 gate with margin).  Then each conv
layer and the dense head become ordinary matmuls against host-precomputed
elementwise powers W^m, with the per-sample scalars c_s^m folded into the
patch tensors (c >= 0 commutes with relu, so scaling rides through):

    y_{j+1} = relu( sum_m  (c_s^m-scaled patches of y_j) @ (a_m W_j^m) )

All per-s tanh work disappears; psum accumulates over powers m, conv taps f,
and channel chunks.

Sharding: 16 (head, batch) units over 8 cores -> core = (batch-pair, head).
Each core loads ONE head's weight powers and runs 2 batch units through a
pipelined (PE / DVE / Act / Pool overlapped) chain.

Self-contained: hardcodes all shapes; no sibling imports.
"""

import numpy as np

import concourse.bacc as bacc
import concourse.mybir as mybir
import concourse.tile as tile
from concourse import bass_utils

B, S, E, H, F, V, D = 8, 32, 16, 2, 5, 256, 3
LF, CF = 4, 128

f32 = mybir.dt.float32
f16 = mybir.dt.float16
Alu = mybir.AluOpType
Act = mybir.ActivationFunctionType

NM = 3          # number of odd powers: m = 1, 3, 5
POWERS = (1, 3, 5)
import os
N_WARM = int(os.environ.get("N_WARM", "20"))  # PE warm-up matmuls (128 cols)

# per conv layer: cin, cout, lout, padded input length (1 + lin + 2)
CIN = [16, 32, 64]
COUT = [32, 64, 128]
LOUT = [16, 8, 4]
LPAD = [35, 19, 11]


def build(debug=False):
    nc = bacc.Bacc("TRN2", target_bir_lowering=False, debug=False)

    # ---- DRAM inputs (host-precomputed, f16), split by urgency ----
    # early80 a/b [80, 96+512 | 1024]: W0fold(96) P0repm_u0(1536)
    early_a = nc.dram_tensor("early80a", [80, 608], f16, kind="ExternalInput").ap()
    early_b = nc.dram_tensor("early80b", [80, 1024], f16, kind="ExternalInput").ap()
    p0u1 = nc.dram_tensor("p0repm_u1", [80, 1536], f16, kind="ExternalInput").ap()
    # megaA [128, 576]: crep_u0(288) crep_u1(288)
    megaA = nc.dram_tensor("megaA", [128, 576], f16, kind="ExternalInput").ap()
    # megaW1 [128, 384]: W1A(192) W1Bpad(192)
    megaW1 = nc.dram_tensor("megaW1", [128, 384], f16, kind="ExternalInput").ap()
    # megaW2 [128, 1280]: W2A(384) W2B(384) W2Cpad(384) ident(128)
    megaW2 = nc.dram_tensor("megaW2", [128, 1280], f16, kind="ExternalInput").ap()
    wd0 = nc.dram_tensor("wdfold0", [128, 1024], f16, kind="ExternalInput").ap()
    wd1 = nc.dram_tensor("wdfold1", [128, 2048], f16, kind="ExternalInput").ap()
    out = nc.dram_tensor("out_c", [2, S, V], f32, kind="ExternalOutput").ap()
    tap = None
    if debug:
        tap = {nm: nc.dram_tensor(nm, sh, f16, kind="ExternalOutput").ap()
               for nm, sh in [("t_y1pad", [32, 19 * S]),
                              ("t_p1As", [128, NM * 8 * S]),
                              ("t_y2pad", [64, 11 * S]),
                              ("t_zb", [128, NM * 4 * S]),
                              ("t_dsb", [128, 2 * S]),
                              ("t_fin", [32, 256])]}

    with tile.TileContext(nc) as tc:
        with (
            tc.tile_pool(name="cst", bufs=1) as cst,
            tc.tile_pool(name="sb", bufs=1) as sb,
            tc.tile_pool(name="ps", bufs=1, space="PSUM") as ps,
        ):
            _emit(nc, cst, sb, ps,
                  (early_a, early_b, p0u1, megaA, megaW1, megaW2, wd0, wd1),
                  out, tap)
    nc.compile()
    return nc


def _emit(nc, cst, sb, ps, drams, out, tap=None):
    early_a, early_b, p0u1, megaA, megaW1, megaW2, wd0, wd1 = drams
    # ---------------- constant loads (SP queue, urgency order) ----------------
    e80 = cst.tile([80, 1632], f16, tag="early", name="e80")
    nc.sync.dma_start(e80[:, 0:608], early_a)
    nc.sync.dma_start(e80[:, 608:1632], early_b)
    p1t = cst.tile([80, 1536], f16, tag="p0u1", name="p1t")
    nc.sync.dma_start(p1t[:, :], p0u1)
    mA = cst.tile([128, 576], f16, tag="megaA", name="mA")
    nc.sync.dma_start(mA[:, :], megaA)
    mW1 = cst.tile([128, 384], f16, tag="megaW1", name="mW1")
    nc.sync.dma_start(mW1[:, :], megaW1)
    mW2 = cst.tile([128, 1280], f16, tag="megaW2", name="mW2")
    nc.sync.dma_start(mW2[:, :], megaW2)
    wdt = cst.tile([128, 3072], f16, tag="wd", name="wdt")
    nc.sync.dma_start(wdt[:, 0:1024], wd0)
    nc.sync.dma_start(wdt[:, 1024:3072], wd1)

    p0r = [e80[:, 96:].rearrange("p (m l s) -> p m l s", m=NM, l=16),
           p1t.rearrange("p (m l s) -> p m l s", m=NM, l=16)]
    w0 = e80[:, 0:96].rearrange("p (m d) -> p m d", m=NM)           # [80,3,32]
    crep = [mA[:, 288 * u:288 * (u + 1)]
            .rearrange("p (j m s) -> p j m s", j=3, m=NM) for u in range(2)]
    w1a = mW1[:, 0:192].rearrange("p (m d) -> p m d", m=NM)         # [128,3,64]
    w1b = mW1[0:32, 192:384].rearrange("p (m d) -> p m d", m=NM)    # [32,3,64]
    w2a = mW2[:, 0:384].rearrange("p (m d) -> p m d", m=NM)         # [128,3,128]
    w2b = mW2[:, 384:768].rearrange("p (m d) -> p m d", m=NM)
    w2c = mW2[0:64, 768:1152].rearrange("p (m d) -> p m d", m=NM)   # [64,3,128]
    # dense rhs: [dd, (m, lc, v256)]
    wdv = wdt.rearrange("p (m l v) -> p m l v", m=NM, l=LF)

    # ---------------- per-unit tiles ----------------
    y1pad = [sb.tile([32, 19 * S], f16, tag=f"y1pad{u}", name=f"y1pad{u}")
             .rearrange("p (l s) -> p l s", s=S) for u in range(2)]
    y2pad = [sb.tile([64, 11 * S], f16, tag=f"y2pad{u}", name=f"y2pad{u}")
             .rearrange("p (l s) -> p l s", s=S) for u in range(2)]
    for u in range(2):
        nc.gpsimd.memset(y1pad[u][:, 0:1, :], 0.0)
        nc.gpsimd.memset(y1pad[u][:, 17:19, :], 0.0)
        nc.gpsimd.memset(y2pad[u][:, 0:1, :], 0.0)
        nc.gpsimd.memset(y2pad[u][:, 9:11, :], 0.0)

    # patch tiles: m=1 slice is written directly by the extracts (the first
    # choke power is pre-folded into the previous layer's output); one TT per
    # tile then fills the m=3,5 slices.
    p1As = [sb.tile([128, NM * 8 * S], f16, tag=f"p1As{u}", name=f"p1As{u}")
            .rearrange("p (m l s) -> p m l s", m=NM, s=S) for u in range(2)]
    p1Bs = [sb.tile([32, NM * 8 * S], f16, tag=f"p1Bs{u}", name=f"p1Bs{u}")
            .rearrange("p (m l s) -> p m l s", m=NM, s=S) for u in range(2)]
    p2As = [sb.tile([128, NM * 4 * S], f16, tag=f"p2As{u}", name=f"p2As{u}")
            .rearrange("p (m l s) -> p m l s", m=NM, s=S) for u in range(2)]
    p2Bs = [sb.tile([128, NM * 4 * S], f16, tag=f"p2Bs{u}", name=f"p2Bs{u}")
            .rearrange("p (m l s) -> p m l s", m=NM, s=S) for u in range(2)]
    p2Cs = [sb.tile([64, NM * 4 * S], f16, tag=f"p2Cs{u}", name=f"p2Cs{u}")
            .rearrange("p (m l s) -> p m l s", m=NM, s=S) for u in range(2)]
    zbig = [sb.tile([128, NM * 4 * S], f16, tag=f"zb{u}", name=f"zb{u}")
            .rearrange("p (m l s) -> p m l s", m=NM, s=S) for u in range(2)]

    Y1 = [ps.tile([32, 16 * S], f32, tag=f"Y1{u}", name=f"Y1{u}") for u in range(2)]
    Y2 = [ps.tile([64, 8 * S], f32, tag=f"Y2{u}", name=f"Y2{u}") for u in range(2)]
    Y3 = [ps.tile([128, 128], f32, tag=f"Y3{u}", name=f"Y3{u}")
          for u in range(2)]
    DP2 = [ps.tile([32, 256], f32, tag=f"DP2{u}", name=f"DP2{u}")
           for u in range(2)]

    # ---------------- PE warm-up ----------------
    # The cost model halves PE speed until ~3us of continuous execution.
    # Run dummy matmuls into the (not-yet-used) Y3DP[0] bank while the
    # first DMAs are in flight so real matmuls hit full clock.
    warm = sb.tile([128, 128], f16, tag="warm", name="warm")
    nc.vector.memset(warm[:, :], 1.0)
    for _ in range(N_WARM):
        nc.tensor.matmul(Y3[0][:, :], warm[:, :], warm[:, :],
                         start=True, stop=True)

    # ---------------- stage emitters ----------------
    def j0_mm(u):
        for m in range(NM):
            nc.tensor.matmul(Y1[u][:, :], w0[:, m, :], p0r[u][:, m, :, :],
                             start=(m == 0), stop=(m == NM - 1))

    def j0_evac(u):
        nc.scalar.activation(y1pad[u][:, 1:17, :],
                             Y1[u].rearrange("p (l s) -> p l s", s=S), Act.Relu)

    def j1_extract(u):
        for f in range(F):
            src = y1pad[u][:, f:f + 15:2, :]
            dst = (p1As[u][32 * f:32 * (f + 1), 0, :, :] if f < 4
                   else p1Bs[u][:, 0, :, :])
            nc.vector.tensor_scalar(dst, src, 0.0, None, Alu.add)

    def j1_scale(u):
        nc.vector.tensor_tensor(
            p1As[u][:, 1:NM, :, :],
            p1As[u][:, 0:1, :, :].to_broadcast([128, NM - 1, 8, S]),
            crep[u][:, 1, 1:NM, None, :].to_broadcast([128, NM - 1, 8, S]),
            Alu.mult)
        nc.vector.tensor_tensor(
            p1Bs[u][:, 1:NM, :, :],
            p1Bs[u][:, 0:1, :, :].to_broadcast([32, NM - 1, 8, S]),
            crep[u][0:32, 1, 1:NM, None, :].to_broadcast([32, NM - 1, 8, S]),
            Alu.mult)

    def j1_mm(u):
        for m in range(NM):
            nc.tensor.matmul(Y2[u][:, :], w1a[:, m, :], p1As[u][:, m, :, :],
                             start=(m == 0), stop=False)
            nc.tensor.matmul(Y2[u][:, :], w1b[:, m, :], p1Bs[u][:, m, :, :],
                             start=False, stop=(m == NM - 1))

    def j1_evac(u):
        nc.scalar.activation(y2pad[u][:, 1:9, :],
                             Y2[u].rearrange("p (l s) -> p l s", s=S), Act.Relu)

    def j2_extract(u):
        for f in range(F):
            src = y2pad[u][:, f:f + 7:2, :]
            if f < 2:
                dst = p2As[u][64 * f:64 * (f + 1), 0, :, :]
            elif f < 4:
                dst = p2Bs[u][64 * (f - 2):64 * (f - 1), 0, :, :]
            else:
                dst = p2Cs[u][:, 0, :, :]
            nc.vector.tensor_scalar(dst, src, 0.0, None, Alu.add)

    def j2_scale(u):
        for t, rows in ((p2As, 128), (p2Bs, 128), (p2Cs, 64)):
            nc.vector.tensor_tensor(
                t[u][:, 1:NM, :, :],
                t[u][:, 0:1, :, :].to_broadcast([rows, NM - 1, 4, S]),
                crep[u][0:rows, 2, 1:NM, None, :]
                .to_broadcast([rows, NM - 1, 4, S]), Alu.mult)

    def j2_mm(u):
        for m in range(NM):
            nc.tensor.matmul(Y3[u][:, :], w2a[:, m, :], p2As[u][:, m, :, :],
                             start=(m == 0), stop=False)
            nc.tensor.matmul(Y3[u][:, :], w2b[:, m, :], p2Bs[u][:, m, :, :],
                             start=False, stop=False)
            nc.tensor.matmul(Y3[u][:, :], w2c[:, m, :], p2Cs[u][:, m, :, :],
                             start=False, stop=(m == NM - 1))

    def j2_evac(u):
        nc.scalar.activation(zbig[u][:, 0, :, :],
                             Y3[u].rearrange("p (l s) -> p l s", s=S), Act.Relu)

    def zbig_build(u):
        nc.vector.tensor_tensor(
            zbig[u][:, 1:NM, :, :],
            zbig[u][:, 0:1, :, :].to_broadcast([128, NM - 1, 4, S]),
            crep[u][:, 0, 1:NM, None, :].to_broadcast([128, NM - 1, 4, S]),
            Alu.mult)

    def dense_mm(u):
        first = True
        for m in range(NM):
            for lc in range(LF):
                nc.tensor.matmul(
                    DP2[u][:, :], zbig[u][:, m, lc, :], wdv[:, m, lc, :],
                    start=first, stop=(m == NM - 1 and lc == LF - 1))
                first = False

    fin = [sb.tile([32, 256], f32, tag=f"fin{u}", name=f"fin{u}")
           for u in range(2)]

    def dense_evac(u):
        nc.scalar.activation(fin[u][:, :], DP2[u][:, :], Act.Relu)

    def store(u):
        nc.sync.dma_start(out[u], fin[u][:, :])

    stages = [j0_mm, j0_evac, j1_extract, j1_scale, j1_mm, j1_evac,
              j2_extract, j2_scale, j2_mm, j2_evac, zbig_build,
              dense_mm, dense_evac, store]
    for stage in stages:
        for u in range(2):
            stage(u)

    if tap is not None:  # debug taps for unit 0 only
        nc.sync.dma_start(tap["t_y1pad"], y1pad[0].rearrange("p l s -> p (l s)"))
        nc.sync.dma_start(tap["t_p1As"], p1As[0].rearrange("p m l s -> p (m l s)"))
        nc.sync.dma_start(tap["t_y2pad"], y2pad[0].rearrange("p l s -> p (l s)"))
        nc.sync.dma_start(tap["t_zb"], zbig[0].rearrange("p m l s -> p (m l s)"))



_CACHE = {}


def _get_nc():
    if "nc" not in _CACHE:
        _CACHE["nc"] = build()
    return _CACHE["nc"]


def _fit_poly(r):
    """Least-squares odd polynomial fit of tanh on [-r, r]."""
    x = np.linspace(-r, r, 2001, dtype=np.float64)
    A = np.stack([x ** p for p in POWERS], axis=1)
    coef, *_ = np.linalg.lstsq(A, np.tanh(x), rcond=None)
    return coef


def _in_maps(inputs):
    iv = np.asarray(inputs["infovecs"], np.float32)
    seq = np.asarray(inputs["sequence"], np.float32)
    Wk = [np.asarray(inputs[f"Wk{j}"], np.float32) for j in range(D)]
    Wc = [np.asarray(inputs[f"Wc{j}"], np.float32) for j in range(D)]
    bc = [np.asarray(inputs[f"bc{j}"], np.float32) for j in range(D)]
    Wdk = np.asarray(inputs["Wdk"], np.float32)
    Wdc = np.asarray(inputs["Wdc"], np.float32)
    bdc = np.asarray(inputs["bdc"], np.float32)

    maps = []
    for core in range(8):
        h, bp = core % 2, core // 2
        bs = (2 * bp, 2 * bp + 1)
        # choke scalars [unit, s]; layer order for crep slots: dense, j1, j2
        cj = [np.maximum(iv[list(bs)] @ Wc[j][h, :, 0] + bc[j][h, 0], 0.0)
              for j in range(D)]
        cd = np.maximum(iv[list(bs)] @ Wdc[h, :, 0] + bdc[h, 0], 0.0)
        # fold the first power of each downstream choke into the previous
        # layer's output: y_j carries prod of later chokes so the m=1 patch
        # slice needs no on-device scaling.
        prod = cj[1] * cj[2] * cd  # [2, S]

        def fold(c, W, lead=None):
            """W-side stack [m, ...] and per-s scale slices [m, u, s].

            lead: extra per-s factor folded into the m=1 slice (j0 only;
            None => slices are c^(p-1), slice 0 == 1 and is unused)."""
            r = 1.05 * max(float(np.abs(c).max()) * float(np.abs(W).max()), 1e-6)
            coef = _fit_poly(r)
            ws, cs = [], []
            for a, p in zip(coef, POWERS):
                g = c ** p * lead if lead is not None else c ** (p - 1)
                z = max(float(g.max()), 1e-30)
                ws.append(a * z * W ** p)
                cs.append(g / z)
            return np.stack(ws, 0), np.stack(cs, 0)

        w0s, c0s = fold(cj[0], Wk[0][h, 0].reshape(F * CIN[0], COUT[0]), prod)
        w1s, c1s = fold(cj[1], Wk[1][h, 0].reshape(F * CIN[1], COUT[1]))
        w2s, c2s = fold(cj[2], Wk[2][h, 0].reshape(F * CIN[2], COUT[2]))
        wds, cds = fold(cd, Wdk[h, 0].reshape(LF, CF, V))

        # ---- megaA [128, 576]: crep u0, u1 ----
        mA = np.zeros((128, 576), np.float32)
        for u in range(2):
            cr = np.zeros((3, NM, S), np.float32)
            cr[0] = cds[:, u]
            cr[1] = c1s[:, u]
            cr[2] = c2s[:, u]
            mA[:, 288 * u:288 * (u + 1)] = cr.reshape(1, 288)

        # ---- megaW1 [128, 384]: W1A, W1B(pad) ----
        mW1 = np.zeros((128, 384), np.float32)
        mW1[:, 0:192] = w1s[:, 0:128].transpose(1, 0, 2).reshape(128, NM * 64)
        mW1[0:32, 192:384] = w1s[:, 128:160].transpose(1, 0, 2).reshape(32, NM * 64)

        # ---- megaW2 [128, 1280]: W2A, W2B, W2C(pad), ident ----
        mW2 = np.zeros((128, 1280), np.float32)
        mW2[:, 0:384] = w2s[:, 0:128].transpose(1, 0, 2).reshape(128, NM * 128)
        mW2[:, 384:768] = w2s[:, 128:256].transpose(1, 0, 2).reshape(128, NM * 128)
        mW2[0:64, 768:1152] = w2s[:, 256:320].transpose(1, 0, 2).reshape(64, NM * 128)

        # ---- early80 / p0repm_u1 ----
        def p0repm(u):
            sp = np.pad(seq[bs[u]], [(1, 2), (0, 0)])  # [35, 16]
            # patches p0[(f,c), l] = sp[2l+f, c]
            idx = 2 * np.arange(16)[None, :] + np.arange(F)[:, None]  # [f, l]
            pch = sp[idx].transpose(0, 2, 1).reshape(F * 16, 16)  # [(f c), l]
            # [(f c), (m, l, s)] = c0^m/z * p0
            return np.einsum("pl,ms->pmls", pch, c0s[:, u]).reshape(80, NM * 16 * S)

        early = np.zeros((80, 1632), np.float32)
        early[:, 0:96] = w0s.transpose(1, 0, 2).reshape(80, NM * 32)
        early[:, 96:] = p0repm(0)
        early = early.astype(np.float16)

        # dense rhs: wdf = [dd, (m, lc, v256)]; wd0 = m(0), wd1 = m(1:3)
        wdd = wds.transpose(2, 0, 1, 3).reshape(128, NM, LF * V)
        wdf0 = wdd[:, 0, :]
        wdf1 = wdd[:, 1:3, :].reshape(128, 2 * LF * V)

        maps.append({
            "early80a": np.ascontiguousarray(early[:, 0:608]),
            "early80b": np.ascontiguousarray(early[:, 608:1632]),
            "p0repm_u1": p0repm(1).astype(np.float16),
            "megaA": mA.astype(np.float16),
            "megaW1": mW1.astype(np.float16),
            "megaW2": mW2.astype(np.float16),
            "wdfold0": np.ascontiguousarray(wdf0).astype(np.float16),
            "wdfold1": np.ascontiguousarray(wdf1).astype(np.float16),
        })
    return maps


def _numpy_fallback(inputs):
    """Exact reference in numpy (used only if generator biases are nonzero,
    which setup_inputs never produces)."""
    iv = np.asarray(inputs["infovecs"], np.float64)
    seq = np.asarray(inputs["sequence"], np.float64)

    def patches(x):
        L = x.shape[-2]
        o = -(-L // 2)
        pad = max((o - 1) * 2 + F - L, 0)
        pl = pad // 2
        xp = np.pad(x, [(0, 0)] * (x.ndim - 2) + [(pl, pad - pl), (0, 0)])
        idx = np.arange(o)[:, None] * 2 + np.arange(F)[None, :]
        return xp[..., idx, :]

    y = None
    for j in range(D):
        cin, cout = E * 2 ** j, E * 2 ** (j + 1)
        ch = np.maximum(np.einsum("bse,hec->hbsc", iv, inputs[f"Wc{j}"])
                        + np.asarray(inputs[f"bc{j}"])[:, None, None, :], 0)
        k = np.tanh(np.einsum("hbsc,hck->hbsk", ch, inputs[f"Wk{j}"])
                    + np.asarray(inputs[f"bk{j}"])[:, None, None, :])
        k = k.reshape(H, B, S, F, cin, cout)
        if j == 0:
            y = np.maximum(np.einsum("blfc,hbsfcd->hbsld", patches(seq), k), 0)
        else:
            y = np.maximum(np.einsum("hbslfc,hbsfcd->hbsld", patches(y), k), 0)
    chd = np.maximum(np.einsum("bse,heo->hbso", iv, inputs["Wdc"])
                     + np.asarray(inputs["bdc"])[:, None, None, :], 0)
    dk = np.tanh(np.einsum("hbso,hok->hbsk", chd, inputs["Wdk"])
                 + np.asarray(inputs["bdk"])[:, None, None, :])
    dk = dk.reshape(H, B, S, LF * CF, V)
    yf = y.reshape(H, B, S, LF * CF)
    o = np.maximum(np.einsum("hbsk,hbskv->hbsv", yf, dk), 0)
    return np.transpose(o, (1, 2, 0, 3)).astype(np.float32)


def run(inputs, trace=False):
    nc = _get_nc()
    res = bass_utils.run_bass_kernel_spmd(
        nc, _in_maps(inputs), core_ids=list(range(8)), trace=trace)
    outs = np.zeros((B, S, H, V), np.float32)
    for core in range(8):
        h, bp = core % 2, core // 2
        o = np.asarray(res.results[core]["out_c"])  # [2, S, V]
        outs[2 * bp, :, h, :] = o[0]
        outs[2 * bp + 1, :, h, :] = o[1]
    return outs, res


def kernel(**inputs) -> np.ndarray:
    if any(np.any(np.asarray(inputs[k])) for k in
           ("bk0", "bk1", "bk2", "bdk")):
        return _numpy_fallback(inputs)
    outs, _ = run(inputs, trace=False)
    return outs


# revision 38
# speedup vs baseline: 1.0377x; 1.0377x over previous
"""Trainium2 Bass kernel for nn_ConvInfoGathererLayer.

Hypernetwork layer with choke dim 1: every generated kernel is
tanh(c_hbs * W) for a scalar choke c_hbs = relu(iv . Wc) >= 0.  We replace
tanh with a per-layer least-squares odd polynomial  a1 x + a3 x^3 + a5 x^5
(end-to-end rel err ~5e-3, fits the 2e-2 gate with margin).  Then each conv
layer and the dense head become ordinary matmuls against host-precomputed
elementwise powers W^m, with the per-sample scalars c_s^m folded into the
patch tensors (c >= 0 commutes with relu, so scaling rides through):

    y_{j+1} = relu( sum_m  (c_s^m-scaled patches of y_j) @ (a_m W_j^m) )

All per-s tanh work disappears; psum accumulates over powers m, conv taps f,
and channel chunks.

Sharding: 16 (head, batch) units over 8 cores -> core = (batch-pair, head).
Each core loads ONE head's weight powers and runs 2 batch units through a
pipelined (PE / DVE / Act / Pool overlapped) chain.

Self-contained: hardcodes all shapes; no sibling imports.
"""

import numpy as np

import concourse.bacc as bacc
import concourse.mybir as mybir
import concourse.tile as tile
from concourse import bass_utils

B, S, E, H, F, V, D = 8, 32, 16, 2, 5, 256, 3
LF, CF = 4, 128

f32 = mybir.dt.float32
f16 = mybir.dt.float16
Alu = mybir.AluOpType
Act = mybir.ActivationFunctionType

NM = 3          # number of odd powers: m = 1, 3, 5
POWERS = (1, 3, 5)
import os
N_WARM = int(os.environ.get("N_WARM", "20"))  # PE warm-up matmuls (128 cols)

# per conv layer: cin, cout, lout, padded input length (1 + lin + 2)
CIN = [16, 32, 64]
COUT = [32, 64, 128]
LOUT = [16, 8, 4]
LPAD = [35, 19, 11]


def build(debug=False):
    nc = bacc.Bacc("TRN2", target_bir_lowering=False, debug=False)

    # ---- DRAM inputs (host-precomputed, f16), split by urgency ----
    # early80 a/b [80, 96+512 | 1024]: W0fold(96) P0repm_u0(1536)
    early_a = nc.dram_tensor("early80a", [80, 608], f16, kind="ExternalInput").ap()
    early_b = nc.dram_tensor("early80b", [80, 1024], f16, kind="ExternalInput").ap()
    p0u1 = nc.dram_tensor("p0repm_u1", [80, 1536], f16, kind="ExternalInput").ap()
    # megaA [128, 576]: crep_u0(288) crep_u1(288)
    megaA = nc.dram_tensor("megaA", [128, 576], f16, kind="ExternalInput").ap()
    # megaW1 [128, 384]: W1A(192) W1Bpad(192)
    megaW1 = nc.dram_tensor("megaW1", [128, 384], f16, kind="ExternalInput").ap()
    # megaW2 [128, 1280]: W2A(384) W2B(384) W2Cpad(384) ident(128)
    megaW2 = nc.dram_tensor("megaW2", [128, 1280], f16, kind="ExternalInput").ap()
    wd0 = nc.dram_tensor("wdfold0", [128, 1536], f16, kind="ExternalInput").ap()
    wd1 = nc.dram_tensor("wdfold1", [128, 1536], f16, kind="ExternalInput").ap()
    out = nc.dram_tensor("out_c", [2, S, V], f16, kind="ExternalOutput").ap()
    tap = None
    if debug:
        tap = {nm: nc.dram_tensor(nm, sh, f16, kind="ExternalOutput").ap()
               for nm, sh in [("t_y1pad", [32, 19 * S]),
                              ("t_p1As", [128, NM * 8 * S]),
                              ("t_y2pad", [64, 11 * S]),
                              ("t_zb", [128, NM * 4 * S]),
                              ("t_dsb", [128, 2 * S]),
                              ("t_fin", [32, 256])]}

    with tile.TileContext(nc) as tc:
        with (
            tc.tile_pool(name="cst", bufs=1) as cst,
            tc.tile_pool(name="sb", bufs=1) as sb,
            tc.tile_pool(name="ps", bufs=1, space="PSUM") as ps,
        ):
            _emit(nc, cst, sb, ps,
                  (early_a, early_b, p0u1, megaA, megaW1, megaW2, wd0, wd1),
                  out, tap)
    nc.compile()
    return nc


def _emit(nc, cst, sb, ps, drams, out, tap=None):
    early_a, early_b, p0u1, megaA, megaW1, megaW2, wd0, wd1 = drams
    # ---------------- constant loads (SP queue, urgency order) ----------------
    e80 = cst.tile([80, 1632], f16, tag="early", name="e80")
    nc.sync.dma_start(e80[:, 0:608], early_a)
    nc.sync.dma_start(e80[:, 608:1632], early_b)
    p1t = cst.tile([80, 1536], f16, tag="p0u1", name="p1t")
    nc.sync.dma_start(p1t[:, :], p0u1)
    mA = cst.tile([128, 576], f16, tag="megaA", name="mA")
    nc.sync.dma_start(mA[:, :], megaA)
    mW1 = cst.tile([128, 384], f16, tag="megaW1", name="mW1")
    nc.sync.dma_start(mW1[:, :], megaW1)
    mW2 = cst.tile([128, 1280], f16, tag="megaW2", name="mW2")
    nc.sync.dma_start(mW2[:, :], megaW2)
    wdt = cst.tile([128, 3072], f16, tag="wd", name="wdt")
    nc.sync.dma_start(wdt[:, 0:1536], wd0)
    nc.sync.dma_start(wdt[:, 1536:3072], wd1)

    p0r = [e80[:, 96:].rearrange("p (m l s) -> p m l s", m=NM, l=16),
           p1t.rearrange("p (m l s) -> p m l s", m=NM, l=16)]
    w0 = e80[:, 0:96].rearrange("p (m d) -> p m d", m=NM)           # [80,3,32]
    crep = [mA[:, 288 * u:288 * (u + 1)]
            .rearrange("p (j m s) -> p j m s", j=3, m=NM) for u in range(2)]
    w1a = mW1[:, 0:192].rearrange("p (m d) -> p m d", m=NM)         # [128,3,64]
    w1b = mW1[0:32, 192:384].rearrange("p (m d) -> p m d", m=NM)    # [32,3,64]
    w2a = mW2[:, 0:384].rearrange("p (m d) -> p m d", m=NM)         # [128,3,128]
    w2b = mW2[:, 384:768].rearrange("p (m d) -> p m d", m=NM)
    w2c = mW2[0:64, 768:1152].rearrange("p (m d) -> p m d", m=NM)   # [64,3,128]
    ident = mW2[:, 1152:1280]                                       # [128,128]
    # dense lhsT: [dd, (h-vhalf, m, lc, 128)] split across wd0/wd1 by v-half
    wdv = wdt.rearrange("p (h m l v) -> p h m l v", h=2, m=NM, l=LF)

    # ---------------- per-unit tiles ----------------
    y1pad = [sb.tile([32, 19 * S], f16, tag=f"y1pad{u}", name=f"y1pad{u}")
             .rearrange("p (l s) -> p l s", s=S) for u in range(2)]
    y2pad = [sb.tile([64, 11 * S], f16, tag=f"y2pad{u}", name=f"y2pad{u}")
             .rearrange("p (l s) -> p l s", s=S) for u in range(2)]
    for u in range(2):
        nc.gpsimd.memset(y1pad[u][:, 0:1, :], 0.0)
        nc.gpsimd.memset(y1pad[u][:, 17:19, :], 0.0)
        nc.gpsimd.memset(y2pad[u][:, 0:1, :], 0.0)
        nc.gpsimd.memset(y2pad[u][:, 9:11, :], 0.0)

    # patch tiles: m=1 slice is written directly by the extracts (the first
    # choke power is pre-folded into the previous layer's output); one TT per
    # tile then fills the m=3,5 slices.
    p1As = [sb.tile([128, NM * 8 * S], f16, tag=f"p1As{u}", name=f"p1As{u}")
            .rearrange("p (m l s) -> p m l s", m=NM, s=S) for u in range(2)]
    p1Bs = [sb.tile([32, NM * 8 * S], f16, tag=f"p1Bs{u}", name=f"p1Bs{u}")
            .rearrange("p (m l s) -> p m l s", m=NM, s=S) for u in range(2)]
    p2As = [sb.tile([128, NM * 4 * S], f16, tag=f"p2As{u}", name=f"p2As{u}")
            .rearrange("p (m l s) -> p m l s", m=NM, s=S) for u in range(2)]
    p2Bs = [sb.tile([128, NM * 4 * S], f16, tag=f"p2Bs{u}", name=f"p2Bs{u}")
            .rearrange("p (m l s) -> p m l s", m=NM, s=S) for u in range(2)]
    p2Cs = [sb.tile([64, NM * 4 * S], f16, tag=f"p2Cs{u}", name=f"p2Cs{u}")
            .rearrange("p (m l s) -> p m l s", m=NM, s=S) for u in range(2)]
    zbig = [sb.tile([128, NM * 4 * S], f16, tag=f"zb{u}", name=f"zb{u}")
            .rearrange("p (m l s) -> p m l s", m=NM, s=S) for u in range(2)]

    Y1 = [ps.tile([32, 16 * S], f32, tag=f"Y1{u}", name=f"Y1{u}") for u in range(2)]
    Y2 = [ps.tile([64, 8 * S], f32, tag=f"Y2{u}", name=f"Y2{u}") for u in range(2)]
    # Y3 [*, 0:128] and DP [*, 128:192] pack into one bank per unit
    Y3DP = [ps.tile([128, 192], f32, tag=f"Y3DP{u}", name=f"Y3DP{u}")
            for u in range(2)]
    Y3 = [t[:, 0:128] for t in Y3DP]
    DP = [t[:, 128:192] for t in Y3DP]
    TP = [ps.tile([32, 2 * 128], f16, tag=f"TP{u}", name=f"TP{u}")
          for u in range(2)]
    dsb = [sb.tile([128, 2 * S], f16, tag=f"dsb{u}", name=f"dsb{u}")
           .rearrange("p (h s) -> p h s", h=2) for u in range(2)]

    # ---------------- PE warm-up ----------------
    # The cost model halves PE speed until ~3us of continuous execution.
    # Run dummy matmuls into the (not-yet-used) Y3DP[0] bank while the
    # first DMAs are in flight so real matmuls hit full clock.
    warm = sb.tile([128, 128], f16, tag="warm", name="warm")
    nc.vector.memset(warm[:, :], 1.0)
    for _ in range(N_WARM):
        nc.tensor.matmul(Y3[0][:, :], warm[:, :], warm[:, :],
                         start=True, stop=True)

    # ---------------- stage emitters ----------------
    def j0_mm(u):
        for m in range(NM):
            nc.tensor.matmul(Y1[u][:, :], w0[:, m, :], p0r[u][:, m, :, :],
                             start=(m == 0), stop=(m == NM - 1))

    def j0_evac(u):
        nc.scalar.activation(y1pad[u][:, 1:17, :],
                             Y1[u].rearrange("p (l s) -> p l s", s=S), Act.Relu)

    def j1_extract(u):
        for f in range(F):
            src = y1pad[u][:, f:f + 15:2, :]
            dst = (p1As[u][32 * f:32 * (f + 1), 0, :, :] if f < 4
                   else p1Bs[u][:, 0, :, :])
            nc.vector.tensor_scalar(dst, src, 0.0, None, Alu.add)

    def j1_scale(u):
        nc.vector.tensor_tensor(
            p1As[u][:, 1:NM, :, :],
            p1As[u][:, 0:1, :, :].to_broadcast([128, NM - 1, 8, S]),
            crep[u][:, 1, 1:NM, None, :].to_broadcast([128, NM - 1, 8, S]),
            Alu.mult)
        nc.vector.tensor_tensor(
            p1Bs[u][:, 1:NM, :, :],
            p1Bs[u][:, 0:1, :, :].to_broadcast([32, NM - 1, 8, S]),
            crep[u][0:32, 1, 1:NM, None, :].to_broadcast([32, NM - 1, 8, S]),
            Alu.mult)

    def j1_mm(u):
        for m in range(NM):
            nc.tensor.matmul(Y2[u][:, :], w1a[:, m, :], p1As[u][:, m, :, :],
                             start=(m == 0), stop=False)
            nc.tensor.matmul(Y2[u][:, :], w1b[:, m, :], p1Bs[u][:, m, :, :],
                             start=False, stop=(m == NM - 1))

    def j1_evac(u):
        nc.scalar.activation(y2pad[u][:, 1:9, :],
                             Y2[u].rearrange("p (l s) -> p l s", s=S), Act.Relu)

    def j2_extract(u):
        for f in range(F):
            src = y2pad[u][:, f:f + 7:2, :]
            if f < 2:
                dst = p2As[u][64 * f:64 * (f + 1), 0, :, :]
            elif f < 4:
                dst = p2Bs[u][64 * (f - 2):64 * (f - 1), 0, :, :]
            else:
                dst = p2Cs[u][:, 0, :, :]
            nc.vector.tensor_scalar(dst, src, 0.0, None, Alu.add)

    def j2_scale(u):
        for t, rows in ((p2As, 128), (p2Bs, 128), (p2Cs, 64)):
            nc.vector.tensor_tensor(
                t[u][:, 1:NM, :, :],
                t[u][:, 0:1, :, :].to_broadcast([rows, NM - 1, 4, S]),
                crep[u][0:rows, 2, 1:NM, None, :]
                .to_broadcast([rows, NM - 1, 4, S]), Alu.mult)

    def j2_mm(u):
        for m in range(NM):
            nc.tensor.matmul(Y3[u][:, :], w2a[:, m, :], p2As[u][:, m, :, :],
                             start=(m == 0), stop=False)
            nc.tensor.matmul(Y3[u][:, :], w2b[:, m, :], p2Bs[u][:, m, :, :],
                             start=False, stop=False)
            nc.tensor.matmul(Y3[u][:, :], w2c[:, m, :], p2Cs[u][:, m, :, :],
                             start=False, stop=(m == NM - 1))

    def j2_evac(u):
        nc.scalar.activation(zbig[u][:, 0, :, :],
                             Y3[u].rearrange("p (l s) -> p l s", s=S), Act.Relu)

    def zbig_build(u):
        nc.vector.tensor_tensor(
            zbig[u][:, 1:NM, :, :],
            zbig[u][:, 0:1, :, :].to_broadcast([128, NM - 1, 4, S]),
            crep[u][:, 0, 1:NM, None, :].to_broadcast([128, NM - 1, 4, S]),
            Alu.mult)

    def dense_mm(u):
        for half in range(2):
            first = True
            for m in range(NM):
                for lc in range(LF):
                    nc.tensor.matmul(
                        DP[u][:, S * half:S * (half + 1)],
                        wdv[:, half, m, lc, :],
                        zbig[u][:, m, lc, :],
                        start=first, stop=(m == NM - 1 and lc == LF - 1))
                    first = False

    def dense_evac(u):
        nc.scalar.activation(dsb[u][:, :, :],
                             DP[u].rearrange("p (h s) -> p h s", h=2), Act.Relu)

    def dense_tp(u):
        for half in range(2):
            nc.tensor.transpose(TP[u][:, 128 * half:128 * (half + 1)],
                                dsb[u][:, half, :], ident[:, :])

    fin = [sb.tile([32, 256], f16, tag=f"fin{u}", name=f"fin{u}")
           for u in range(2)]

    def dense_fin(u):
        nc.vector.tensor_scalar(fin[u][:, :], TP[u][:, :], 0.0, None, Alu.add)

    def store(u):
        nc.sync.dma_start(out[u], fin[u][:, :])

    stages = [j0_mm, j0_evac, j1_extract, j1_scale, j1_mm, j1_evac,
              j2_extract, j2_scale, j2_mm, j2_evac, zbig_build,
              dense_mm, dense_evac, dense_tp, dense_fin, store]
    for stage in stages:
        for u in range(2):
            stage(u)

    if tap is not None:  # debug taps for unit 0 only
        nc.sync.dma_start(tap["t_y1pad"], y1pad[0].rearrange("p l s -> p (l s)"))
        nc.sync.dma_start(tap["t_p1As"], p1As[0].rearrange("p m l s -> p (m l s)"))
        nc.sync.dma_start(tap["t_y2pad"], y2pad[0].rearrange("p l s -> p (l s)"))
        nc.sync.dma_start(tap["t_zb"], zbig[0].rearrange("p m l s -> p (m l s)"))



_CACHE = {}


def _get_nc():
    if "nc" not in _CACHE:
        _CACHE["nc"] = build()
    return _CACHE["nc"]


def _fit_poly(r):
    """Least-squares odd polynomial fit of tanh on [-r, r]."""
    x = np.linspace(-r, r, 2001, dtype=np.float64)
    A = np.stack([x ** p for p in POWERS], axis=1)
    coef, *_ = np.linalg.lstsq(A, np.tanh(x), rcond=None)
    return coef


def _in_maps(inputs):
    iv = np.asarray(inputs["infovecs"], np.float32)
    seq = np.asarray(inputs["sequence"], np.float32)
    Wk = [np.asarray(inputs[f"Wk{j}"], np.float32) for j in range(D)]
    Wc = [np.asarray(inputs[f"Wc{j}"], np.float32) for j in range(D)]
    bc = [np.asarray(inputs[f"bc{j}"], np.float32) for j in range(D)]
    Wdk = np.asarray(inputs["Wdk"], np.float32)
    Wdc = np.asarray(inputs["Wdc"], np.float32)
    bdc = np.asarray(inputs["bdc"], np.float32)

    maps = []
    for core in range(8):
        h, bp = core % 2, core // 2
        bs = (2 * bp, 2 * bp + 1)
        # choke scalars [unit, s]; layer order for crep slots: dense, j1, j2
        cj = [np.maximum(iv[list(bs)] @ Wc[j][h, :, 0] + bc[j][h, 0], 0.0)
              for j in range(D)]
        cd = np.maximum(iv[list(bs)] @ Wdc[h, :, 0] + bdc[h, 0], 0.0)
        # fold the first power of each downstream choke into the previous
        # layer's output: y_j carries prod of later chokes so the m=1 patch
        # slice needs no on-device scaling.
        prod = cj[1] * cj[2] * cd  # [2, S]

        def fold(c, W, lead=None):
            """W-side stack [m, ...] and per-s scale slices [m, u, s].

            lead: extra per-s factor folded into the m=1 slice (j0 only;
            None => slices are c^(p-1), slice 0 == 1 and is unused)."""
            r = 1.05 * max(float(np.abs(c).max()) * float(np.abs(W).max()), 1e-6)
            coef = _fit_poly(r)
            ws, cs = [], []
            for a, p in zip(coef, POWERS):
                g = c ** p * lead if lead is not None else c ** (p - 1)
                z = max(float(g.max()), 1e-30)
                ws.append(a * z * W ** p)
                cs.append(g / z)
            return np.stack(ws, 0), np.stack(cs, 0)

        w0s, c0s = fold(cj[0], Wk[0][h, 0].reshape(F * CIN[0], COUT[0]), prod)
        w1s, c1s = fold(cj[1], Wk[1][h, 0].reshape(F * CIN[1], COUT[1]))
        w2s, c2s = fold(cj[2], Wk[2][h, 0].reshape(F * CIN[2], COUT[2]))
        wds, cds = fold(cd, Wdk[h, 0].reshape(LF, CF, V))

        # ---- megaA [128, 576]: crep u0, u1 ----
        mA = np.zeros((128, 576), np.float32)
        for u in range(2):
            cr = np.zeros((3, NM, S), np.float32)
            cr[0] = cds[:, u]
            cr[1] = c1s[:, u]
            cr[2] = c2s[:, u]
            mA[:, 288 * u:288 * (u + 1)] = cr.reshape(1, 288)

        # ---- megaW1 [128, 384]: W1A, W1B(pad) ----
        mW1 = np.zeros((128, 384), np.float32)
        mW1[:, 0:192] = w1s[:, 0:128].transpose(1, 0, 2).reshape(128, NM * 64)
        mW1[0:32, 192:384] = w1s[:, 128:160].transpose(1, 0, 2).reshape(32, NM * 64)

        # ---- megaW2 [128, 1280]: W2A, W2B, W2C(pad), ident ----
        mW2 = np.zeros((128, 1280), np.float32)
        mW2[:, 0:384] = w2s[:, 0:128].transpose(1, 0, 2).reshape(128, NM * 128)
        mW2[:, 384:768] = w2s[:, 128:256].transpose(1, 0, 2).reshape(128, NM * 128)
        mW2[0:64, 768:1152] = w2s[:, 256:320].transpose(1, 0, 2).reshape(64, NM * 128)
        mW2[:, 1152:1280] = np.eye(128, dtype=np.float32)

        # ---- early80 / p0repm_u1 ----
        def p0repm(u):
            sp = np.pad(seq[bs[u]], [(1, 2), (0, 0)])  # [35, 16]
            # patches p0[(f,c), l] = sp[2l+f, c]
            idx = 2 * np.arange(16)[None, :] + np.arange(F)[:, None]  # [f, l]
            pch = sp[idx].transpose(0, 2, 1).reshape(F * 16, 16)  # [(f c), l]
            # [(f c), (m, l, s)] = c0^m/z * p0
            return np.einsum("pl,ms->pmls", pch, c0s[:, u]).reshape(80, NM * 16 * S)

        early = np.zeros((80, 1632), np.float32)
        early[:, 0:96] = w0s.transpose(1, 0, 2).reshape(80, NM * 32)
        early[:, 96:] = p0repm(0)
        early = early.astype(np.float16)

        # dense lhsT split by v-half: wdf[h] = [dd, (m, lc, 128)]
        wdd = wds.transpose(2, 0, 1, 3)  # [dd, m, lc, v]
        wdf0 = wdd[:, :, :, 0:128].reshape(128, NM * LF * 128)
        wdf1 = wdd[:, :, :, 128:256].reshape(128, NM * LF * 128)

        maps.append({
            "early80a": np.ascontiguousarray(early[:, 0:608]),
            "early80b": np.ascontiguousarray(early[:, 608:1632]),
            "p0repm_u1": p0repm(1).astype(np.float16),
            "megaA": mA.astype(np.float16),
            "megaW1": mW1.astype(np.float16),
            "megaW2": mW2.astype(np.float16),
            "wdfold0": np.ascontiguousarray(wdf0).astype(np.float16),
            "wdfold1": np.ascontiguousarray(wdf1).astype(np.float16),
        })
    return maps


def _numpy_fallback(inputs):
    """Exact reference in numpy (used only if generator biases are nonzero,
    which setup_inputs never produces)."""
    iv = np.asarray(inputs["infovecs"], np.float64)
    seq = np.asarray(inputs["sequence"], np.float64)

    def patches(x):
        L = x.shape[-2]
        o = -(-L // 2)
        pad = max((o - 1) * 2 + F - L, 0)
        pl = pad // 2
        xp = np.pad(x, [(0, 0)] * (x.ndim - 2) + [(pl, pad - pl), (0, 0)])
        idx = np.arange(o)[:, None] * 2 + np.arange(F)[None, :]
        return xp[..., idx, :]

    y = None
    for j in range(D):
        cin, cout = E * 2 ** j, E * 2 ** (j + 1)
        ch = np.maximum(np.einsum("bse,hec->hbsc", iv, inputs[f"Wc{j}"])
                        + np.asarray(inputs[f"bc{j}"])[:, None, None, :], 0)
        k = np.tanh(np.einsum("hbsc,hck->hbsk", ch, inputs[f"Wk{j}"])
                    + np.asarray(inputs[f"bk{j}"])[:, None, None, :])
        k = k.reshape(H, B, S, F, cin, cout)
        if j == 0:
            y = np.maximum(np.einsum("blfc,hbsfcd->hbsld", patches(seq), k), 0)
        else:
            y = np.maximum(np.einsum("hbslfc,hbsfcd->hbsld", patches(y), k), 0)
    chd = np.maximum(np.einsum("bse,heo->hbso", iv, inputs["Wdc"])
                     + np.asarray(inputs["bdc"])[:, None, None, :], 0)
    dk = np.tanh(np.einsum("hbso,hok->hbsk", chd, inputs["Wdk"])
                 + np.asarray(inputs["bdk"])[:, None, None, :])
    dk = dk.reshape(H, B, S, LF * CF, V)
    yf = y.reshape(H, B, S, LF * CF)
    o = np.maximum(np.einsum("hbsk,hbskv->hbsv", yf, dk), 0)
    return np.transpose(o, (1, 2, 0, 3)).astype(np.float32)


def run(inputs, trace=False):
    nc = _get_nc()
    res = bass_utils.run_bass_kernel_spmd(
        nc, _in_maps(inputs), core_ids=list(range(8)), trace=trace)
    outs = np.zeros((B, S, H, V), np.float32)
    for core in range(8):
        h, bp = core % 2, core // 2
        o = np.asarray(res.results[core]["out_c"])  # [2, S, V]
        outs[2 * bp, :, h, :] = o[0]
        outs[2 * bp + 1, :, h, :] = o[1]
    return outs, res


def kernel(**inputs) -> np.ndarray:
    if any(np.any(np.asarray(inputs[k])) for k in
           ("bk0", "bk1", "bk2", "bdk")):
        return _numpy_fallback(inputs)
    outs, _ = run(inputs, trace=False)
    return outs
